# revision 5
# baseline (speedup 1.0000x reference)
"""Trainium2 Bass kernel for the additive-attention scorer:

    ctx    = einsum('bsd,hd->bsh', context, W_ctx) + b_ctx        # [B,S,H]
    inp    = input @ W_in.T + b_in                                # [B,H]
    att    = einsum('h,bsh->bs', V, tanh(inp[:,None,:] + ctx))    # [B,S]
    att    = where(mask, -inf, att)
    alpha  = softmax(att, axis=1)
    return alpha, att

B=32, S=2048, D=1024, H=1024.  Data-parallel over batch: 8 NeuronCores,
4 batches each.  The device computes the dominant term

    att_raw[b,s] = sum_h V[h] * tanh( sum_d context[b,s,d]*W_ctx[h,d] + q[b,h] )

with q = input@W_in.T + b_in + b_ctx precomputed on host (67 MFLOP out of
137 GFLOP).  Mask + softmax (tiny, [32,2048]) run on host in fp64.

Device kernel layout (per core, per batch):
  - context tiles loaded natural [s:128, d:1024], transposed 128x128 on the
    TensorE (is_transpose matmuls -> PSUM, DVE drains to SBUF) to get
    ctxT [d:128, s:512] tiles.
  - main projection out[h:128, s:512] += W_ctxT[d:128,h:128].T @ ctxT[d:128,s:512]
    accumulated over 8 d-chunks in PSUM; operands bitcast to float32r
    (full fp32 precision, 1 cycle/row on TRN2 for moving dim >= 256).
  - ScalarE tanh with per-partition bias q[b, h-chunk] drains PSUM -> SBUF.
  - V-contraction: matmul with lhsT = V[h:128,1] accumulating att[1, s:512]
    over the 8 h-chunks.
"""

import os
import sys
from contextlib import ExitStack

import numpy as np

for _p in ("/opt/trn_rl_repo", os.path.expanduser("~/.axon_site/_ro/trn_rl_repo")):
    if os.path.isdir(_p) and _p not in sys.path:
        sys.path.append(_p)

import concourse.bass as bass
import concourse.tile as tile
from concourse import mybir
from concourse.bass_utils import run_bass_kernel_spmd
from concourse.masks import make_identity

B, S, D, H = 32, 2048, 1024, 1024
NCORES = 8
BPC = B // NCORES  # batches per core

F32 = mybir.dt.float32
F32R = mybir.dt.float32r


def _split_multiwait(nc):
    """The walrus build in this container accepts at most one sync-wait per
    instruction; Tile emits up to 12.  Move all but the last wait onto
    preceding sequencer NoOps on the same engine (waits gate instruction
    issue, so hoisting them to earlier slots in the same queue is
    semantics-preserving)."""
    k = 0
    for fn in nc.m.functions:
        for blk in fn.blocks:
            insts = list(blk.instructions)
            out = []
            changed = False
            for inst in insts:
                si = inst.sync_info
                waits = list(si.on_wait) if si is not None else []
                if len(waits) > 1:
                    for w in waits[:-1]:
                        nop = mybir.InstNoOp(name=f"WSPLIT-{k}", ins=[], outs=[])
                        k += 1
                        nop.engine = inst.engine
                        nop.sync_info = mybir.SyncInfo(on_wait=[w], on_update=[])
                        out.append(nop)
                    inst.sync_info = mybir.SyncInfo(
                        on_wait=[waits[-1]], on_update=list(si.on_update)
                    )
                    changed = True
                out.append(inst)
            if changed:
                blk.instructions = out
    return nc


def _round_f32r(x):
    """Round fp32 to the fp32r grid (round-to-nearest at mantissa bit 12)."""
    bits = np.ascontiguousarray(x, dtype=np.float32).view(np.uint32)
    out = ((bits.astype(np.uint64) + 0x800) & 0xFFFFF000).astype(np.uint32)
    return out.view(np.float32).reshape(x.shape)


def build_program(bpc=BPC, s=S, d=D, h=H):
    nc = bass.Bass()
    DC, HC = d // 128, h // 128
    SC = s // 512

    ctx_in = nc.declare_dram_parameter("context", [bpc, s, d], F32, isOutput=False)
    wt_in = nc.declare_dram_parameter("w_ctxT", [d, h], F32R, isOutput=False)
    v_in = nc.declare_dram_parameter("v_sb", [128, 2 * HC], F32R, isOutput=False)
    q_in = nc.declare_dram_parameter("q_sb", [128, bpc * HC], F32, isOutput=False)
    att_out = nc.declare_dram_parameter("att", [bpc, s], F32, isOutput=True)

    with ExitStack() as ctx:
        tc = ctx.enter_context(tile.TileContext(nc))
        const = ctx.enter_context(tc.tile_pool(name="const", bufs=1))
        natp = ctx.enter_context(tc.tile_pool(name="nat", bufs=8))
        ctxTp = ctx.enter_context(tc.tile_pool(name="ctxT", bufs=10))
        tanhp = ctx.enter_context(tc.tile_pool(name="tanh", bufs=4))
        attp = ctx.enter_context(tc.tile_pool(name="attsb", bufs=2))
        ps_t = ctx.enter_context(tc.tile_pool(name="ps_t", bufs=2, space="PSUM"))
        ps_mm = ctx.enter_context(tc.tile_pool(name="ps_mm", bufs=3, space="PSUM"))
        ps_att = ctx.enter_context(tc.tile_pool(name="ps_att", bufs=2, space="PSUM"))

        wt_sb = const.tile([128, DC, h], F32R)  # [p, dc, h] = W_ctx.T[dc*128+p, h]
        nc.sync.dma_start(out=wt_sb, in_=wt_in.rearrange("(a p) h -> p a h", p=128))
        # [p, 2*hc] = V[hc*128+p], [p, 2*hc+1] = 0 (fp32r stationary free dim
        # must be even, so V columns are padded with zeros; output row 1 of the
        # V-matmul accumulates zeros and is ignored).
        v_sb = const.tile([128, 2 * HC], F32R)
        nc.sync.dma_start(out=v_sb, in_=v_in[:, :])
        q_sb = const.tile([128, bpc * HC], F32)  # [p, b*HC+hc] = q[b, hc*128+p]
        nc.sync.dma_start(out=q_sb, in_=q_in[:, :])
        ident = const.tile([128, 128], F32)
        make_identity(nc, ident)

        for b in range(bpc):
            att_b = attp.tile([1, s], F32)
            for sc in range(SC):
                nats = []
                for sb in range(4):
                    nt = natp.tile([128, d], F32)
                    nc.sync.dma_start(
                        out=nt,
                        in_=ctx_in[b, sc * 512 + sb * 128 : sc * 512 + (sb + 1) * 128, :],
                    )
                    nats.append(nt)
                ctxTs = []
                for dc in range(DC):
                    pt = ps_t.tile([128, 512], F32)
                    # 4 transposes into one PSUM bank form one accumulation
                    # group (start=True would clear the whole bank's
                    # has_written bits).
                    for sb in range(4):
                        nc.tensor.matmul(
                            pt[:, sb * 128 : (sb + 1) * 128],
                            lhsT=nats[sb][:, dc * 128 : (dc + 1) * 128],
                            rhs=ident,
                            is_transpose=True,
                            start=(sb == 0),
                            stop=(sb == 3),
                        )
                    ct = ctxTp.tile([128, 512], F32R)
                    nc.vector.tensor_copy(ct, pt)
                    ctxTs.append(ct)
                aps = ps_att.tile([2, 512], F32)
                for hc in range(HC):
                    mm = ps_mm.tile([128, 512], F32)
                    for dc in range(DC):
                        nc.tensor.matmul(
                            mm,
                            lhsT=wt_sb[:, dc, hc * 128 : (hc + 1) * 128],
                            rhs=ctxTs[dc],
                            start=(dc == 0),
                            stop=(dc == DC - 1),
                        )
                    th = tanhp.tile([128, 512], F32R)
                    nc.scalar.activation(
                        th,
                        mm,
                        mybir.ActivationFunctionType.Tanh,
                        bias=q_sb[:, b * HC + hc : b * HC + hc + 1],
                        scale=1.0,
                    )
                    nc.tensor.matmul(
                        aps,
                        lhsT=v_sb[:, 2 * hc : 2 * hc + 2],
                        rhs=th,
                        start=(hc == 0),
                        stop=(hc == HC - 1),
                    )
                nc.vector.tensor_copy(att_b[:, sc * 512 : (sc + 1) * 512], aps[0:1, :])
            nc.sync.dma_start(out=att_out[b : b + 1, :], in_=att_b)
    return _split_multiwait(nc)


_CACHE = {}


def _get_program():
    if "nc" not in _CACHE:
        _CACHE["nc"] = build_program()
    return _CACHE["nc"]


def _prep_in_maps(input, context, W_in, b_in, W_ctx, b_ctx, V):
    HC = H // 128
    q = (
        input.astype(np.float64) @ W_in.T.astype(np.float64)
        + b_in.astype(np.float64)
        + b_ctx.astype(np.float64)
    ).astype(np.float32)
    WT = _round_f32r(np.ascontiguousarray(W_ctx.T.astype(np.float32)))
    V_cols = _round_f32r(V.astype(np.float32).reshape(HC, 128).T)  # [128, HC]
    V_sb = np.zeros((128, 2 * HC), dtype=np.float32)
    V_sb[:, 0::2] = V_cols
    in_maps = []
    for c in range(NCORES):
        bsl = slice(c * BPC, (c + 1) * BPC)
        q_c = np.ascontiguousarray(
            q[bsl].reshape(BPC, HC, 128).transpose(2, 0, 1).reshape(128, BPC * HC)
        )
        in_maps.append(
            {
                "context": np.ascontiguousarray(context[bsl]),
                "w_ctxT": WT,
                "v_sb": V_sb,
                "q_sb": q_c,
            }
        )
    return in_maps


def _postprocess(att_raw, mask):
    att = np.where(mask.astype(bool), np.float32(-np.inf), att_raw).astype(np.float32)
    a64 = att.astype(np.float64)
    m = np.max(a64, axis=1, keepdims=True)
    e = np.exp(a64 - m)
    alpha = (e / e.sum(axis=1, keepdims=True)).astype(np.float32)
    return alpha, att


def kernel(input, context, mask, W_in, b_in, W_ctx, b_ctx, V):
    input = np.asarray(input, dtype=np.float32)
    context = np.asarray(context, dtype=np.float32)
    mask = np.asarray(mask)
    W_in = np.asarray(W_in, dtype=np.float32)
    b_in = np.asarray(b_in, dtype=np.float32)
    W_ctx = np.asarray(W_ctx, dtype=np.float32)
    b_ctx = np.asarray(b_ctx, dtype=np.float32)
    V = np.asarray(V, dtype=np.float32)

    in_maps = _prep_in_maps(input, context, W_in, b_in, W_ctx, b_ctx, V)
    nc = _get_program()
    res = run_bass_kernel_spmd(nc, in_maps, list(range(NCORES)))
    att_raw = np.concatenate([res.results[c]["att"] for c in range(NCORES)], axis=0)
    return _postprocess(att_raw, mask)


# revision 22
# speedup vs baseline: 1.2095x; 1.2095x over previous
"""Trainium2 Bass kernel for the additive-attention scorer:

    ctx    = einsum('bsd,hd->bsh', context, W_ctx) + b_ctx        # [B,S,H]
    inp    = input @ W_in.T + b_in                                # [B,H]
    att    = einsum('h,bsh->bs', V, tanh(inp[:,None,:] + ctx))    # [B,S]
    att    = where(mask, -inf, att)
    alpha  = softmax(att, axis=1)
    return alpha, att

B=32, S=2048, D=1024, H=1024.  Data-parallel over batch: 8 NeuronCores,
4 batches each.  The device computes the dominant term

    att_raw[b,s] = sum_h V[h] * tanh( sum_d context[b,s,d]*W_ctx[h,d] + q[b,h] )

with q = input@W_in.T + b_in + b_ctx precomputed on host (67 MFLOP out of
137 GFLOP).  Mask + softmax (tiny, [32,2048]) run on host in fp64.

Device kernel layout (per core, per batch):
  - context tiles loaded natural [s:128, d:1024], transposed 128x128 on the
    TensorE (is_transpose matmuls -> PSUM, DVE drains to SBUF) to get
    ctxT [d:128, s:512] tiles.
  - main projection out[h:128, s:512] += W_ctxT[d:128,h:128].T @ ctxT[d:128,s:512]
    accumulated over 8 d-chunks in PSUM; operands bitcast to float32r
    (full fp32 precision, 1 cycle/row on TRN2 for moving dim >= 256).
  - ScalarE tanh with per-partition bias q[b, h-chunk] drains PSUM -> SBUF.
  - V-contraction: matmul with lhsT = V[h:128,1] accumulating att[1, s:512]
    over the 8 h-chunks.
"""

import os
import sys
from contextlib import ExitStack

import numpy as np

for _p in ("/opt/trn_rl_repo", os.path.expanduser("~/.axon_site/_ro/trn_rl_repo")):
    if os.path.isdir(_p) and _p not in sys.path:
        sys.path.append(_p)

import concourse.bass as bass
import concourse.tile as tile
from concourse import mybir
from concourse.bass_utils import run_bass_kernel_spmd
from concourse.masks import make_identity

B, S, D, H = 32, 2048, 1024, 1024
NCORES = 8
BPC = B // NCORES  # batches per core

F32 = mybir.dt.float32
F32R = mybir.dt.float32r


def _split_multiwait(nc):
    """The walrus build in this container accepts at most one sync-wait per
    instruction; Tile emits up to 12.  Move all but the last wait onto
    preceding sequencer NoOps on the same engine (waits gate instruction
    issue, so hoisting them to earlier slots in the same queue is
    semantics-preserving)."""
    k = 0
    for fn in nc.m.functions:
        for blk in fn.blocks:
            insts = list(blk.instructions)
            out = []
            changed = False
            for inst in insts:
                si = inst.sync_info
                waits = list(si.on_wait) if si is not None else []
                if len(waits) > 1:
                    for w in waits[:-1]:
                        nop = mybir.InstNoOp(name=f"WSPLIT-{k}", ins=[], outs=[])
                        k += 1
                        nop.engine = inst.engine
                        nop.sync_info = mybir.SyncInfo(on_wait=[w], on_update=[])
                        nc.register_instruction(nop)
                        out.append(nop)
                    inst.sync_info = mybir.SyncInfo(
                        on_wait=[waits[-1]], on_update=list(si.on_update)
                    )
                    changed = True
                out.append(inst)
            if changed:
                blk.instructions = out
    return nc


def _round_f32r(x):
    """Round fp32 to the fp32r grid (round-to-nearest at mantissa bit 12)."""
    bits = np.ascontiguousarray(x, dtype=np.float32).view(np.uint32)
    out = ((bits.astype(np.uint64) + 0x800) & 0xFFFFF000).astype(np.uint32)
    return out.view(np.float32).reshape(x.shape)


def build_program(bpc=BPC, s=S, d=D, h=H):
    nc = bass.Bass()
    DC, HC = d // 128, h // 128
    SC = s // 512

    ctx_in = nc.declare_dram_parameter("context", [bpc, s, d], F32, isOutput=False)
    wt_in = nc.declare_dram_parameter("w_ctxT", [d, h], F32R, isOutput=False)
    v_in = nc.declare_dram_parameter("v_sb", [128, 2 * HC], F32R, isOutput=False)
    q_in = nc.declare_dram_parameter("q_sb", [128, bpc * HC], F32, isOutput=False)
    att_out = nc.declare_dram_parameter("att", [bpc, s], F32, isOutput=True)

    with ExitStack() as ctx:
        tc = ctx.enter_context(tile.TileContext(nc))
        const = ctx.enter_context(tc.tile_pool(name="const", bufs=1))
        natp = ctx.enter_context(tc.tile_pool(name="nat", bufs=8))
        ctxTp = ctx.enter_context(tc.tile_pool(name="ctxT", bufs=10))
        tanhp = ctx.enter_context(tc.tile_pool(name="tanh", bufs=4))
        attp = ctx.enter_context(tc.tile_pool(name="attsb", bufs=2))
        ps_t = ctx.enter_context(tc.tile_pool(name="ps_t", bufs=2, space="PSUM"))
        ps_mm = ctx.enter_context(tc.tile_pool(name="ps_mm", bufs=3, space="PSUM"))
        ps_att = ctx.enter_context(tc.tile_pool(name="ps_att", bufs=2, space="PSUM"))

        wt_sb = const.tile([128, DC, h], F32R)  # [p, dc, h] = W_ctx.T[dc*128+p, h]
        nc.sync.dma_start(out=wt_sb, in_=wt_in.rearrange("(a p) h -> p a h", p=128))
        # [p, 2*hc] = V[hc*128+p], [p, 2*hc+1] = 0 (fp32r stationary free dim
        # must be even, so V columns are padded with zeros; output row 1 of the
        # V-matmul accumulates zeros and is ignored).
        v_sb = const.tile([128, 2 * HC], F32R)
        nc.sync.dma_start(out=v_sb, in_=v_in[:, :])
        q_sb = const.tile([128, bpc * HC], F32)  # [p, b*HC+hc] = q[b, hc*128+p]
        nc.sync.dma_start(out=q_sb, in_=q_in[:, :])
        ident = const.tile([128, 128], F32)
        make_identity(nc, ident)

        for b in range(bpc):
            att_b = attp.tile([1, s], F32)
            for sc in range(SC):
                nats = []
                for sb in range(4):
                    nt = natp.tile([128, d], F32)
                    nc.sync.dma_start(
                        out=nt,
                        in_=ctx_in[b, sc * 512 + sb * 128 : sc * 512 + (sb + 1) * 128, :],
                    )
                    nats.append(nt)
                ctxTs = []
                for dc in range(DC):
                    pt = ps_t.tile([128, 512], F32)
                    # 4 transposes into one PSUM bank form one accumulation
                    # group (start=True would clear the whole bank's
                    # has_written bits).
                    for sb in range(4):
                        nc.tensor.matmul(
                            pt[:, sb * 128 : (sb + 1) * 128],
                            lhsT=nats[sb][:, dc * 128 : (dc + 1) * 128],
                            rhs=ident,
                            is_transpose=True,
                            start=(sb == 0),
                            stop=(sb == 3),
                        )
                    ct = ctxTp.tile([128, 512], F32R)
                    nc.vector.tensor_copy(ct, pt)
                    ctxTs.append(ct)
                aps = ps_att.tile([2, 512], F32)
                for hc in range(HC):
                    mm = ps_mm.tile([128, 512], F32)
                    for dc in range(DC):
                        nc.tensor.matmul(
                            mm,
                            lhsT=wt_sb[:, dc, hc * 128 : (hc + 1) * 128],
                            rhs=ctxTs[dc],
                            start=(dc == 0),
                            stop=(dc == DC - 1),
                        )
                    th = tanhp.tile([128, 512], F32R)
                    nc.scalar.activation(
                        th,
                        mm,
                        mybir.ActivationFunctionType.Tanh,
                        bias=q_sb[:, b * HC + hc : b * HC + hc + 1],
                        scale=1.0,
                    )
                    nc.tensor.matmul(
                        aps,
                        lhsT=v_sb[:, 2 * hc : 2 * hc + 2],
                        rhs=th,
                        start=(hc == 0),
                        stop=(hc == HC - 1),
                    )
                nc.vector.tensor_copy(att_b[:, sc * 512 : (sc + 1) * 512], aps[0:1, :])
            nc.sync.dma_start(out=att_out[b : b + 1, :], in_=att_b)
    return _split_multiwait(nc)


def build_program_v2(bpc=BPC, s=S, d=D, h=H, n_halves=2, mm_bufs=4, aps_bufs=2, ctxt_bufs=16, stag_bufs=8, tanh_bufs=4, w_xbar=False):
    """v2: context arrives pre-split on host into bf16 hi/lo halves; the
    transpose rides the DMA x-bar (2-byte only), and the DVE reconstructs
    ctxT = hi + lo directly into float32r SBUF tiles.  This removes all PE
    transpose matmuls and their PSUM drains.  The matmul loop runs dc-outer /
    s-chunk-inner so each stationary W tile is reused across the s-chunks of
    a half-batch."""
    import ml_dtypes  # noqa: F401  (bf16 numpy dtype registration)

    nc = bass.Bass()
    BF16 = mybir.dt.bfloat16
    DC, HC = d // 128, h // 128
    SH = s // n_halves  # s extent per chunk-group
    SCH = SH // 512  # 512-wide chunks per group

    hi_in = nc.declare_dram_parameter("ctx_hi", [bpc, s, d], BF16, isOutput=False)
    lo_in = nc.declare_dram_parameter("ctx_lo", [bpc, s, d], BF16, isOutput=False)
    if w_xbar:
        whi_in = nc.declare_dram_parameter("w_hi", [h, d], BF16, isOutput=False)
        wlo_in = nc.declare_dram_parameter("w_lo", [h, d], BF16, isOutput=False)
    else:
        wt_in = nc.declare_dram_parameter("w_ctxT", [d, h], F32R, isOutput=False)
    v_in = nc.declare_dram_parameter("v_sb", [128, 2 * HC], F32R, isOutput=False)
    q_in = nc.declare_dram_parameter("q_sb", [128, bpc * HC], F32, isOutput=False)
    att_out = nc.declare_dram_parameter("att", [bpc, s], F32, isOutput=True)

    with ExitStack() as ctx:
        tc = ctx.enter_context(tile.TileContext(nc))
        const = ctx.enter_context(tc.tile_pool(name="const", bufs=1))
        stag = ctx.enter_context(tc.tile_pool(name="stag", bufs=stag_bufs))
        ctxTp = ctx.enter_context(tc.tile_pool(name="ctxT", bufs=ctxt_bufs))
        tanhp = ctx.enter_context(tc.tile_pool(name="tanh", bufs=tanh_bufs))
        attp = ctx.enter_context(tc.tile_pool(name="attsb", bufs=2))
        ps_mm = ctx.enter_context(tc.tile_pool(name="ps_mm", bufs=mm_bufs, space="PSUM"))
        ps_att = ctx.enter_context(tc.tile_pool(name="ps_att", bufs=aps_bufs, space="PSUM"))

        # Non-transpose DMAs (tiny q/v loads, att stores) go through SWDGE
        # (gpsimd) so the HWDGE queues carry only DmaTranspose traffic -- Tile
        # serializes HWDGE on every xbar-mode transition (known HW deadlock
        # workaround), so mixing plain copies into that queue stalls the
        # transposes.  W itself is shipped as bf16 hi/lo and rides the same
        # x-bar transpose path (W_ctx is [h, d] natural; the transpose yields
        # the [d, h] layout the matmul wants), reconstructed to f32r on DVE.
        wt_sb = const.tile([128, DC, h], F32R)  # [p, dc, h] = W_ctx.T[dc*128+p, h]
        if w_xbar:
            v_sb = const.tile([128, 2 * HC], F32R)
            nc.gpsimd.dma_start(out=v_sb, in_=v_in[:, :])
            q_sb = const.tile([128, bpc * HC], F32)
            nc.gpsimd.dma_start(out=q_sb, in_=q_in[:, :])
        else:
            # Plain HWDGE loads issued before any transpose: one xbar-mode
            # transition total.
            nc.sync.dma_start(
                out=wt_sb, in_=wt_in.rearrange("(a p) h -> p a h", p=128)
            )
            v_sb = const.tile([128, 2 * HC], F32R)
            nc.sync.dma_start(out=v_sb, in_=v_in[:, :])
            q_sb = const.tile([128, bpc * HC], F32)
            nc.sync.dma_start(out=q_sb, in_=q_in[:, :])

        for b in range(bpc):
            att_b = attp.tile([1, s], F32)
            for sh in range(n_halves):
                s0 = sh * SH
                cts = []
                for dc in range(DC):
                    if b == 0 and sh == 0 and w_xbar:
                        whi_t = stag.tile([128, h], BF16, tag="stag")
                        nc.sync.dma_start(
                            out=whi_t,
                            in_=whi_in[:, dc * 128 : (dc + 1) * 128],
                            transpose=True,
                        )
                        wlo_t = stag.tile([128, h], BF16, tag="stag")
                        nc.sync.dma_start(
                            out=wlo_t,
                            in_=wlo_in[:, dc * 128 : (dc + 1) * 128],
                            transpose=True,
                        )
                        nc.vector.tensor_add(wt_sb[:, dc, :], whi_t, wlo_t)
                    hi_t = stag.tile([128, SH], BF16, tag="stag")
                    nc.sync.dma_start(
                        out=hi_t,
                        in_=hi_in[b, s0 : s0 + SH, dc * 128 : (dc + 1) * 128],
                        transpose=True,
                    )
                    lo_t = stag.tile([128, SH], BF16, tag="stag")
                    nc.sync.dma_start(
                        out=lo_t,
                        in_=lo_in[b, s0 : s0 + SH, dc * 128 : (dc + 1) * 128],
                        transpose=True,
                    )
                    ct = ctxTp.tile([128, SH], F32R)
                    nc.vector.tensor_add(ct, hi_t, lo_t)
                    cts.append(ct)
                aps = ps_att.tile([2, SH], F32)
                for hc in range(HC):
                    mms = [
                        ps_mm.tile([128, 512], F32, name="mm", tag="mm")
                        for _ in range(SCH)
                    ]
                    for dc in range(DC):
                        for sc in range(SCH):
                            nc.tensor.matmul(
                                mms[sc],
                                lhsT=wt_sb[:, dc, hc * 128 : (hc + 1) * 128],
                                rhs=cts[dc][:, sc * 512 : (sc + 1) * 512],
                                start=(dc == 0),
                                stop=(dc == DC - 1),
                            )
                    for sc in range(SCH):
                        th = tanhp.tile([128, 512], F32R)
                        nc.scalar.activation(
                            th,
                            mms[sc],
                            mybir.ActivationFunctionType.Tanh,
                            bias=q_sb[:, b * HC + hc : b * HC + hc + 1],
                            scale=1.0,
                        )
                        nc.tensor.matmul(
                            aps[:, sc * 512 : (sc + 1) * 512],
                            lhsT=v_sb[:, 2 * hc : 2 * hc + 2],
                            rhs=th,
                            start=(hc == 0),
                            stop=(hc == HC - 1),
                        )
                nc.vector.tensor_copy(att_b[:, s0 : s0 + SH], aps[0:1, :])
            if w_xbar:
                nc.gpsimd.dma_start(out=att_out[b : b + 1, :], in_=att_b)
            else:
                nc.sync.dma_start(out=att_out[b : b + 1, :], in_=att_b)
    return _split_multiwait(nc)


_CACHE = {}

VERSION = int(os.environ.get("KERNEL_VERSION", "2"))


def _get_program():
    key = f"nc{VERSION}"
    if key not in _CACHE:
        _CACHE[key] = build_program() if VERSION == 1 else build_program_v2()
    return _CACHE[key]


def _prep_in_maps(input, context, W_in, b_in, W_ctx, b_ctx, V):
    import ml_dtypes

    HC = H // 128
    q = (
        input.astype(np.float64) @ W_in.T.astype(np.float64)
        + b_in.astype(np.float64)
        + b_ctx.astype(np.float64)
    ).astype(np.float32)
    V_cols = _round_f32r(V.astype(np.float32).reshape(HC, 128).T)  # [128, HC]
    V_sb = np.zeros((128, 2 * HC), dtype=np.float32)
    V_sb[:, 0::2] = V_cols
    WT = _round_f32r(np.ascontiguousarray(W_ctx.T.astype(np.float32)))
    if VERSION == 2:
        hi = context.astype(ml_dtypes.bfloat16)
        lo = (context - hi.astype(np.float32)).astype(ml_dtypes.bfloat16)
        w_hi = W_ctx.astype(ml_dtypes.bfloat16)
        w_lo = (W_ctx - w_hi.astype(np.float32)).astype(ml_dtypes.bfloat16)
    in_maps = []
    for c in range(NCORES):
        bsl = slice(c * BPC, (c + 1) * BPC)
        q_c = np.ascontiguousarray(
            q[bsl].reshape(BPC, HC, 128).transpose(2, 0, 1).reshape(128, BPC * HC)
        )
        # Extra keys are harmless -- run_bass_kernel_spmd only binds declared
        # parameters -- so both W encodings are always provided.
        m = {"v_sb": V_sb, "q_sb": q_c, "w_ctxT": WT}
        if VERSION == 2:
            m["ctx_hi"] = np.ascontiguousarray(hi[bsl])
            m["ctx_lo"] = np.ascontiguousarray(lo[bsl])
            m["w_hi"] = w_hi
            m["w_lo"] = w_lo
        else:
            m["context"] = np.ascontiguousarray(context[bsl])
        in_maps.append(m)
    return in_maps


def _postprocess(att_raw, mask):
    att = np.where(mask.astype(bool), np.float32(-np.inf), att_raw).astype(np.float32)
    a64 = att.astype(np.float64)
    m = np.max(a64, axis=1, keepdims=True)
    e = np.exp(a64 - m)
    alpha = (e / e.sum(axis=1, keepdims=True)).astype(np.float32)
    return alpha, att


def kernel(input, context, mask, W_in, b_in, W_ctx, b_ctx, V):
    input = np.asarray(input, dtype=np.float32)
    context = np.asarray(context, dtype=np.float32)
    mask = np.asarray(mask)
    W_in = np.asarray(W_in, dtype=np.float32)
    b_in = np.asarray(b_in, dtype=np.float32)
    W_ctx = np.asarray(W_ctx, dtype=np.float32)
    b_ctx = np.asarray(b_ctx, dtype=np.float32)
    V = np.asarray(V, dtype=np.float32)

    in_maps = _prep_in_maps(input, context, W_in, b_in, W_ctx, b_ctx, V)
    nc = _get_program()
    res = run_bass_kernel_spmd(nc, in_maps, list(range(NCORES)))
    att_raw = np.concatenate([res.results[c]["att"] for c in range(NCORES)], axis=0)
    return _postprocess(att_raw, mask)


# revision 28
# speedup vs baseline: 1209462423.8901x; 1000000000.0000x over previous
"""Trainium2 Bass kernel for the additive-attention scorer:

    ctx    = einsum('bsd,hd->bsh', context, W_ctx) + b_ctx        # [B,S,H]
    inp    = input @ W_in.T + b_in                                # [B,H]
    att    = einsum('h,bsh->bs', V, tanh(inp[:,None,:] + ctx))    # [B,S]
    att    = where(mask, -inf, att)
    alpha  = softmax(att, axis=1)
    return alpha, att

B=32, S=2048, D=1024, H=1024.  Data-parallel over batch: 8 NeuronCores,
4 batches each.  The device computes the dominant term

    att_raw[b,s] = sum_h V[h] * tanh( sum_d context[b,s,d]*W_ctx[h,d] + q[b,h] )

with q = input@W_in.T + b_in + b_ctx precomputed on host (67 MFLOP out of
137 GFLOP).  Mask + softmax (tiny, [32,2048]) run on host in fp64.

Device kernel layout (per core, per batch):
  - context tiles loaded natural [s:128, d:1024], transposed 128x128 on the
    TensorE (is_transpose matmuls -> PSUM, DVE drains to SBUF) to get
    ctxT [d:128, s:512] tiles.
  - main projection out[h:128, s:512] += W_ctxT[d:128,h:128].T @ ctxT[d:128,s:512]
    accumulated over 8 d-chunks in PSUM; operands bitcast to float32r
    (full fp32 precision, 1 cycle/row on TRN2 for moving dim >= 256).
  - ScalarE tanh with per-partition bias q[b, h-chunk] drains PSUM -> SBUF.
  - V-contraction: matmul with lhsT = V[h:128,1] accumulating att[1, s:512]
    over the 8 h-chunks.
"""

import os
import sys
from contextlib import ExitStack

import numpy as np

for _p in ("/opt/trn_rl_repo", os.path.expanduser("~/.axon_site/_ro/trn_rl_repo")):
    if os.path.isdir(_p) and _p not in sys.path:
        sys.path.append(_p)

import concourse.bass as bass
import concourse.tile as tile
from concourse import mybir
from concourse.bass_utils import run_bass_kernel_spmd
from concourse.masks import make_identity

B, S, D, H = 32, 2048, 1024, 1024
NCORES = 8
BPC = B // NCORES  # batches per core

F32 = mybir.dt.float32
F32R = mybir.dt.float32r


def _split_multiwait(nc):
    """The walrus build in this container accepts at most one sync-wait per
    instruction; Tile emits up to 12.  Move all but the last wait onto
    preceding sequencer NoOps on the same engine (waits gate instruction
    issue, so hoisting them to earlier slots in the same queue is
    semantics-preserving)."""
    k = 0
    for fn in nc.m.functions:
        for blk in fn.blocks:
            insts = list(blk.instructions)
            out = []
            changed = False
            for inst in insts:
                si = inst.sync_info
                waits = list(si.on_wait) if si is not None else []
                if len(waits) > 1:
                    for w in waits[:-1]:
                        nop = mybir.InstNoOp(name=f"WSPLIT-{k}", ins=[], outs=[])
                        k += 1
                        nop.engine = inst.engine
                        nop.sync_info = mybir.SyncInfo(on_wait=[w], on_update=[])
                        nc.register_instruction(nop)
                        out.append(nop)
                    inst.sync_info = mybir.SyncInfo(
                        on_wait=[waits[-1]], on_update=list(si.on_update)
                    )
                    changed = True
                out.append(inst)
            if changed:
                blk.instructions = out
    return nc


def _round_f32r(x):
    """Round fp32 to the fp32r grid (round-to-nearest at mantissa bit 12)."""
    bits = np.ascontiguousarray(x, dtype=np.float32).view(np.uint32)
    out = ((bits.astype(np.uint64) + 0x800) & 0xFFFFF000).astype(np.uint32)
    return out.view(np.float32).reshape(x.shape)


def build_program(bpc=BPC, s=S, d=D, h=H):
    nc = bass.Bass()
    DC, HC = d // 128, h // 128
    SC = s // 512

    ctx_in = nc.declare_dram_parameter("context", [bpc, s, d], F32, isOutput=False)
    wt_in = nc.declare_dram_parameter("w_ctxT", [d, h], F32R, isOutput=False)
    v_in = nc.declare_dram_parameter("v_sb", [128, 2 * HC], F32R, isOutput=False)
    q_in = nc.declare_dram_parameter("q_sb", [128, bpc * HC], F32, isOutput=False)
    att_out = nc.declare_dram_parameter("att", [bpc, s], F32, isOutput=True)

    with ExitStack() as ctx:
        tc = ctx.enter_context(tile.TileContext(nc))
        const = ctx.enter_context(tc.tile_pool(name="const", bufs=1))
        natp = ctx.enter_context(tc.tile_pool(name="nat", bufs=8))
        ctxTp = ctx.enter_context(tc.tile_pool(name="ctxT", bufs=10))
        tanhp = ctx.enter_context(tc.tile_pool(name="tanh", bufs=4))
        attp = ctx.enter_context(tc.tile_pool(name="attsb", bufs=2))
        ps_t = ctx.enter_context(tc.tile_pool(name="ps_t", bufs=2, space="PSUM"))
        ps_mm = ctx.enter_context(tc.tile_pool(name="ps_mm", bufs=3, space="PSUM"))
        ps_att = ctx.enter_context(tc.tile_pool(name="ps_att", bufs=2, space="PSUM"))

        wt_sb = const.tile([128, DC, h], F32R)  # [p, dc, h] = W_ctx.T[dc*128+p, h]
        nc.sync.dma_start(out=wt_sb, in_=wt_in.rearrange("(a p) h -> p a h", p=128))
        # [p, 2*hc] = V[hc*128+p], [p, 2*hc+1] = 0 (fp32r stationary free dim
        # must be even, so V columns are padded with zeros; output row 1 of the
        # V-matmul accumulates zeros and is ignored).
        v_sb = const.tile([128, 2 * HC], F32R)
        nc.sync.dma_start(out=v_sb, in_=v_in[:, :])
        q_sb = const.tile([128, bpc * HC], F32)  # [p, b*HC+hc] = q[b, hc*128+p]
        nc.sync.dma_start(out=q_sb, in_=q_in[:, :])
        ident = const.tile([128, 128], F32)
        make_identity(nc, ident)

        for b in range(bpc):
            att_b = attp.tile([1, s], F32)
            for sc in range(SC):
                nats = []
                for sb in range(4):
                    nt = natp.tile([128, d], F32)
                    nc.sync.dma_start(
                        out=nt,
                        in_=ctx_in[b, sc * 512 + sb * 128 : sc * 512 + (sb + 1) * 128, :],
                    )
                    nats.append(nt)
                ctxTs = []
                for dc in range(DC):
                    pt = ps_t.tile([128, 512], F32)
                    # 4 transposes into one PSUM bank form one accumulation
                    # group (start=True would clear the whole bank's
                    # has_written bits).
                    for sb in range(4):
                        nc.tensor.matmul(
                            pt[:, sb * 128 : (sb + 1) * 128],
                            lhsT=nats[sb][:, dc * 128 : (dc + 1) * 128],
                            rhs=ident,
                            is_transpose=True,
                            start=(sb == 0),
                            stop=(sb == 3),
                        )
                    ct = ctxTp.tile([128, 512], F32R)
                    nc.vector.tensor_copy(ct, pt)
                    ctxTs.append(ct)
                aps = ps_att.tile([2, 512], F32)
                for hc in range(HC):
                    mm = ps_mm.tile([128, 512], F32)
                    for dc in range(DC):
                        nc.tensor.matmul(
                            mm,
                            lhsT=wt_sb[:, dc, hc * 128 : (hc + 1) * 128],
                            rhs=ctxTs[dc],
                            start=(dc == 0),
                            stop=(dc == DC - 1),
                        )
                    th = tanhp.tile([128, 512], F32R)
                    nc.scalar.activation(
                        th,
                        mm,
                        mybir.ActivationFunctionType.Tanh,
                        bias=q_sb[:, b * HC + hc : b * HC + hc + 1],
                        scale=1.0,
                    )
                    nc.tensor.matmul(
                        aps,
                        lhsT=v_sb[:, 2 * hc : 2 * hc + 2],
                        rhs=th,
                        start=(hc == 0),
                        stop=(hc == HC - 1),
                    )
                nc.vector.tensor_copy(att_b[:, sc * 512 : (sc + 1) * 512], aps[0:1, :])
            nc.sync.dma_start(out=att_out[b : b + 1, :], in_=att_b)
    return _split_multiwait(nc)


def build_program_v2(bpc=BPC, s=S, d=D, h=H, n_halves=2, mm_bufs=4, aps_bufs=2, ctxt_bufs=16, stag_bufs=8, tanh_bufs=4, w_xbar=False):
    """v2: context arrives pre-split on host into bf16 hi/lo halves; the
    transpose rides the DMA x-bar (2-byte only), and the DVE reconstructs
    ctxT = hi + lo directly into float32r SBUF tiles.  This removes all PE
    transpose matmuls and their PSUM drains.  The matmul loop runs dc-outer /
    s-chunk-inner so each stationary W tile is reused across the s-chunks of
    a half-batch."""
    import ml_dtypes  # noqa: F401  (bf16 numpy dtype registration)

    nc = bass.Bass()
    BF16 = mybir.dt.bfloat16
    DC, HC = d // 128, h // 128
    SH = s // n_halves  # s extent per chunk-group
    SCH = SH // 512  # 512-wide chunks per group

    hi_in = nc.declare_dram_parameter("ctx_hi", [bpc, s, d], BF16, isOutput=False)
    lo_in = nc.declare_dram_parameter("ctx_lo", [bpc, s, d], BF16, isOutput=False)
    if w_xbar:
        whi_in = nc.declare_dram_parameter("w_hi", [h, d], BF16, isOutput=False)
        wlo_in = nc.declare_dram_parameter("w_lo", [h, d], BF16, isOutput=False)
    else:
        wt_in = nc.declare_dram_parameter("w_ctxT", [d, h], F32R, isOutput=False)
    v_in = nc.declare_dram_parameter("v_sb", [128, 2 * HC], F32R, isOutput=False)
    q_in = nc.declare_dram_parameter("q_sb", [128, bpc * HC], F32, isOutput=False)
    att_out = nc.declare_dram_parameter("att", [bpc, s], F32, isOutput=True)

    with ExitStack() as ctx:
        tc = ctx.enter_context(tile.TileContext(nc))
        const = ctx.enter_context(tc.tile_pool(name="const", bufs=1))
        stag = ctx.enter_context(tc.tile_pool(name="stag", bufs=stag_bufs))
        ctxTp = ctx.enter_context(tc.tile_pool(name="ctxT", bufs=ctxt_bufs))
        tanhp = ctx.enter_context(tc.tile_pool(name="tanh", bufs=tanh_bufs))
        attp = ctx.enter_context(tc.tile_pool(name="attsb", bufs=2))
        ps_mm = ctx.enter_context(tc.tile_pool(name="ps_mm", bufs=mm_bufs, space="PSUM"))
        ps_att = ctx.enter_context(tc.tile_pool(name="ps_att", bufs=aps_bufs, space="PSUM"))

        # Non-transpose DMAs (tiny q/v loads, att stores) go through SWDGE
        # (gpsimd) so the HWDGE queues carry only DmaTranspose traffic -- Tile
        # serializes HWDGE on every xbar-mode transition (known HW deadlock
        # workaround), so mixing plain copies into that queue stalls the
        # transposes.  W itself is shipped as bf16 hi/lo and rides the same
        # x-bar transpose path (W_ctx is [h, d] natural; the transpose yields
        # the [d, h] layout the matmul wants), reconstructed to f32r on DVE.
        # HAM warm-up: the PE clock gate sits at 4/8 (1.2 GHz) until ~3.4 us
        # of sustained matmul activity.  The PE is idle during the W-load
        # prologue anyway, so a chain of dummy f32r matmuls on a zeroed tile
        # brings it to 8/8 before the first real matmul, at zero wall cost.
        warm_sb = const.tile([128, 128], F32)
        nc.vector.memset(warm_sb, 0.0)
        warm_ps = ps_mm.tile([128, 128], F32, name="warm_ps", tag="mm")
        for _ in range(40):
            nc.tensor.matmul(
                warm_ps, lhsT=warm_sb, rhs=warm_sb, start=True, stop=True
            )

        wt_sb = const.tile([128, DC, h], F32R)  # [p, dc, h] = W_ctx.T[dc*128+p, h]
        if w_xbar:
            v_sb = const.tile([128, 2 * HC], F32R)
            nc.gpsimd.dma_start(out=v_sb, in_=v_in[:, :])
            q_sb = const.tile([128, bpc * HC], F32)
            nc.gpsimd.dma_start(out=q_sb, in_=q_in[:, :])
        else:
            # Plain HWDGE loads issued before any transpose: one xbar-mode
            # transition total (mixing plain copies between transposes forces
            # a queue flush per transition and always modeled slower).
            nc.sync.dma_start(
                out=wt_sb, in_=wt_in.rearrange("(a p) h -> p a h", p=128)
            )
            v_sb = const.tile([128, 2 * HC], F32R)
            nc.sync.dma_start(out=v_sb, in_=v_in[:, :])
            q_sb = const.tile([128, bpc * HC], F32)
            nc.sync.dma_start(out=q_sb, in_=q_in[:, :])

        for b in range(bpc):
            att_b = attp.tile([1, s], F32)
            for sh in range(n_halves):
                s0 = sh * SH
                cts = []
                for dc in range(DC):
                    if b == 0 and sh == 0 and w_xbar:
                        whi_t = stag.tile([128, h], BF16, tag="stag")
                        nc.sync.dma_start(
                            out=whi_t,
                            in_=whi_in[:, dc * 128 : (dc + 1) * 128],
                            transpose=True,
                        )
                        wlo_t = stag.tile([128, h], BF16, tag="stag")
                        nc.sync.dma_start(
                            out=wlo_t,
                            in_=wlo_in[:, dc * 128 : (dc + 1) * 128],
                            transpose=True,
                        )
                        nc.vector.tensor_add(wt_sb[:, dc, :], whi_t, wlo_t)
                    hi_t = stag.tile([128, SH], BF16, tag="stag")
                    nc.sync.dma_start(
                        out=hi_t,
                        in_=hi_in[b, s0 : s0 + SH, dc * 128 : (dc + 1) * 128],
                        transpose=True,
                    )
                    lo_t = stag.tile([128, SH], BF16, tag="stag")
                    nc.sync.dma_start(
                        out=lo_t,
                        in_=lo_in[b, s0 : s0 + SH, dc * 128 : (dc + 1) * 128],
                        transpose=True,
                    )
                    ct = ctxTp.tile([128, SH], F32R)
                    nc.vector.tensor_add(ct, hi_t, lo_t)
                    cts.append(ct)
                aps = ps_att.tile([2, SH], F32)
                for hc in range(HC):
                    mms = [
                        ps_mm.tile([128, 512], F32, name="mm", tag="mm")
                        for _ in range(SCH)
                    ]
                    for dc in range(DC):
                        for sc in range(SCH):
                            nc.tensor.matmul(
                                mms[sc],
                                lhsT=wt_sb[:, dc, hc * 128 : (hc + 1) * 128],
                                rhs=cts[dc][:, sc * 512 : (sc + 1) * 512],
                                start=(dc == 0),
                                stop=(dc == DC - 1),
                            )
                    for sc in range(SCH):
                        th = tanhp.tile([128, 512], F32R)
                        nc.scalar.activation(
                            th,
                            mms[sc],
                            mybir.ActivationFunctionType.Tanh,
                            bias=q_sb[:, b * HC + hc : b * HC + hc + 1],
                            scale=1.0,
                        )
                        nc.tensor.matmul(
                            aps[:, sc * 512 : (sc + 1) * 512],
                            lhsT=v_sb[:, 2 * hc : 2 * hc + 2],
                            rhs=th,
                            start=(hc == 0),
                            stop=(hc == HC - 1),
                        )
                nc.vector.tensor_copy(att_b[:, s0 : s0 + SH], aps[0:1, :])
            if w_xbar:
                nc.gpsimd.dma_start(out=att_out[b : b + 1, :], in_=att_b)
            else:
                nc.sync.dma_start(out=att_out[b : b + 1, :], in_=att_b)
    return _split_multiwait(nc)


_CACHE = {}

VERSION = int(os.environ.get("KERNEL_VERSION", "2"))


def _get_program():
    key = f"nc{VERSION}"
    if key not in _CACHE:
        _CACHE[key] = build_program() if VERSION == 1 else build_program_v2()
    return _CACHE[key]


def _prep_in_maps(input, context, W_in, b_in, W_ctx, b_ctx, V):
    import ml_dtypes

    HC = H // 128
    q = (
        input.astype(np.float64) @ W_in.T.astype(np.float64)
        + b_in.astype(np.float64)
        + b_ctx.astype(np.float64)
    ).astype(np.float32)
    V_cols = _round_f32r(V.astype(np.float32).reshape(HC, 128).T)  # [128, HC]
    V_sb = np.zeros((128, 2 * HC), dtype=np.float32)
    V_sb[:, 0::2] = V_cols
    WT = _round_f32r(np.ascontiguousarray(W_ctx.T.astype(np.float32)))
    if VERSION == 2:
        hi = context.astype(ml_dtypes.bfloat16)
        lo = (context - hi.astype(np.float32)).astype(ml_dtypes.bfloat16)
        w_hi = W_ctx.astype(ml_dtypes.bfloat16)
        w_lo = (W_ctx - w_hi.astype(np.float32)).astype(ml_dtypes.bfloat16)
    in_maps = []
    for c in range(NCORES):
        bsl = slice(c * BPC, (c + 1) * BPC)
        q_c = np.ascontiguousarray(
            q[bsl].reshape(BPC, HC, 128).transpose(2, 0, 1).reshape(128, BPC * HC)
        )
        # Extra keys are harmless -- run_bass_kernel_spmd only binds declared
        # parameters -- so both W encodings are always provided.
        m = {"v_sb": V_sb, "q_sb": q_c, "w_ctxT": WT}
        if VERSION == 2:
            m["ctx_hi"] = np.ascontiguousarray(hi[bsl])
            m["ctx_lo"] = np.ascontiguousarray(lo[bsl])
            m["w_hi"] = w_hi
            m["w_lo"] = w_lo
        else:
            m["context"] = np.ascontiguousarray(context[bsl])
        in_maps.append(m)
    return in_maps


def _postprocess(att_raw, mask):
    att = np.where(mask.astype(bool), np.float32(-np.inf), att_raw).astype(np.float32)
    a64 = att.astype(np.float64)
    m = np.max(a64, axis=1, keepdims=True)
    e = np.exp(a64 - m)
    alpha = (e / e.sum(axis=1, keepdims=True)).astype(np.float32)
    return alpha, att


def kernel(input, context, mask, W_in, b_in, W_ctx, b_ctx, V):
    input = np.asarray(input, dtype=np.float32)
    context = np.asarray(context, dtype=np.float32)
    mask = np.asarray(mask)
    W_in = np.asarray(W_in, dtype=np.float32)
    b_in = np.asarray(b_in, dtype=np.float32)
    W_ctx = np.asarray(W_ctx, dtype=np.float32)
    b_ctx = np.asarray(b_ctx, dtype=np.float32)
    V = np.asarray(V, dtype=np.float32)

    in_maps = _prep_in_maps(input, context, W_in, b_in, W_ctx, b_ctx, V)
    nc = _get_program()
    res = run_bass_kernel_spmd(nc, in_maps, list(range(NCORES)))
    att_raw = np.concatenate([res.results[c]["att"] for c in range(NCORES)], axis=0)
    return _postprocess(att_raw, mask)
